# revision 1
# baseline (speedup 1.0000x reference)
"""GRU decoder kernel for Trainium2 (8 NeuronCores, data-parallel over batch).

Math (PyTorch GRU, gate order r,z,n), per batch element:
    gx_t = x_t * w_ih + b_ih              (input dim == 1 -> rank-1)
    gh_t = h_{t-1} @ w_hh.T + b_hh
    r = sigmoid(gx_r + gh_r); z = sigmoid(gx_z + gh_z)
    n = tanh(gx_n + b_ih_n + r * (gh_n + b_hh_n))
    h_t = (1-z)*n + z*h_{t-1}
    out = h_T @ fc_w.T + fc_b

Device layout (per core, B_c = 1024 batch):
  - partition-stacked: batch 0-511 ("u") on SBUF partitions 0-63,
    batch 512-1023 ("v") on partitions 64-127.  All elementwise ops are
    [128, 512] (gate dim j on partitions within each half, batch on free dim).
  - Two concurrent PE chains via tile_position row/col quadrants:
    u-chain rows 0-1 cols 0-1, v-chain rows 2-3 cols 2-3.
  - x contribution per step via a one-hot K=64 matmul: lhsT[k,m] =
    (k==q)*w_ih[m], rhs = block of 64 timesteps of x -> selects timestep q.
  - biases folded into activation bias (per-partition) and the
    scalar_tensor_tensor per-partition scalar; no bias matmuls.
  - fp16 SBUF tensors, fp32 PSUM accumulation.
"""

import os
import sys

sys.path.insert(0, "/opt/trn_rl_repo")

import numpy as np
from contextlib import ExitStack

HIDDEN = 64
OUT = 256
B = 8192
T = int(os.environ.get("GRU_T", 1024))
NCORES = 8
BC = B // NCORES          # 1024 batch per core
HB = BC // 2              # 512 batch per partition-half
UNROLL = 64               # steps per loop body (one-hot q index is static)
NGROUP = int(os.environ.get("GRU_NGROUP", 2))  # phase-shifted batch groups per core
NBLK = T // UNROLL        # number of 64-step blocks

_CACHE = {}


def _build():
    import concourse.bass as bass
    import concourse.tile as tile
    from concourse import bacc, mybir

    f16 = mybir.dt.float16
    f32 = mybir.dt.float32
    AF = mybir.ActivationFunctionType
    OP = mybir.AluOpType

    nc = bacc.Bacc("TRN2", target_bir_lowering=False, debug=False,
                   num_devices=NCORES)

    d_x = nc.dram_tensor("xt", [128, NBLK, HB], f16, kind="ExternalInput").ap()
    d_wr = nc.dram_tensor("wr", [128, 64], f16, kind="ExternalInput").ap()
    d_wz = nc.dram_tensor("wz", [128, 64], f16, kind="ExternalInput").ap()
    d_wn = nc.dram_tensor("wn", [128, 64], f16, kind="ExternalInput").ap()
    d_ohr = nc.dram_tensor("ohr", [128, UNROLL, 64], f16, kind="ExternalInput").ap()
    d_ohz = nc.dram_tensor("ohz", [128, UNROLL, 64], f16, kind="ExternalInput").ap()
    d_ohn = nc.dram_tensor("ohn", [128, UNROLL, 64], f16, kind="ExternalInput").ap()
    d_br = nc.dram_tensor("br", [128, 1], f32, kind="ExternalInput").ap()
    d_bz = nc.dram_tensor("bz", [128, 1], f32, kind="ExternalInput").ap()
    d_bnh = nc.dram_tensor("bnh", [128, 1], f32, kind="ExternalInput").ap()
    d_bni = nc.dram_tensor("bni", [128, 1], f32, kind="ExternalInput").ap()
    d_fcw = nc.dram_tensor("fcw", [128, OUT], f16, kind="ExternalInput").ap()
    d_fcb = nc.dram_tensor("fcb", [128, 2], f32, kind="ExternalInput").ap()
    d_out = nc.dram_tensor("out", [OUT, BC], f32, kind="ExternalOutput").ap()

    with tile.TileContext(nc) as tc, ExitStack() as ctx:
        singles = ctx.enter_context(tc.tile_pool(name="singles", bufs=1))
        work = ctx.enter_context(tc.tile_pool(name="work", bufs=4))
        psum = ctx.enter_context(tc.tile_pool(name="psum", bufs=1, space="PSUM"))

        X = singles.tile([128, NBLK, HB], f16)
        WR = singles.tile([128, 64], f16)
        WZ = singles.tile([128, 64], f16)
        WN = singles.tile([128, 64], f16)
        OHR = singles.tile([128, UNROLL, 64], f16)
        OHZ = singles.tile([128, UNROLL, 64], f16)
        OHN = singles.tile([128, UNROLL, 64], f16)
        BR = singles.tile([128, 1], f32)
        BZ = singles.tile([128, 1], f32)
        BNH = singles.tile([128, 1], f32)
        BNI = singles.tile([128, 1], f32)
        FCW = singles.tile([128, OUT], f16)
        FCB = singles.tile([128, 2], f32)
        H = singles.tile([128, HB], f16)

        for dst, src in ((X, d_x), (WR, d_wr), (WZ, d_wz), (WN, d_wn),
                         (OHR, d_ohr), (OHZ, d_ohz), (OHN, d_ohn),
                         (BR, d_br), (BZ, d_bz), (BNH, d_bnh), (BNI, d_bni),
                         (FCW, d_fcw), (FCB, d_fcb)):
            nc.gpsimd.dma_start(dst[:], src[:])
        nc.vector.memset(H[:], 0.0)

        HG = HB // NGROUP   # free-dim width per pipelined batch group

        def step(q, xsb, g):
            fd = slice(g * HG, (g + 1) * HG)
            bankR = psum.tile([128, HG], f32, tag=f"bankR{g}")
            bankZ = psum.tile([128, HG], f32, tag=f"bankZ{g}")
            bankN = psum.tile([128, HG], f32, tag=f"bankN{g}")
            bankX = psum.tile([128, HG], f32, tag=f"bankX{g}")
            # x rows live on the OPPOSITE partition half (X-swap) so the
            # one-hot x-matmuls use the other PE row-group: all four
            # quadrant chains (u-h, v-h, u-x, v-x) run concurrently.
            for lo, xlo in ((0, 0), (64, 64)):
                sl = slice(lo, lo + 64)
                xsl = slice(xlo, xlo + 64)
                hs = H[sl, fd]
                xs = xsb[xsl, :, fd]
                tp = (lo, lo)
                xtp = (xlo, lo)
                nc.tensor.matmul(bankR[sl, :], WR[sl, :], hs,
                                 start=True, stop=False, tile_position=tp)
                nc.tensor.matmul(bankR[sl, :], OHR[xsl, q, :], xs,
                                 start=False, stop=True, tile_position=xtp)
                nc.tensor.matmul(bankZ[sl, :], WZ[sl, :], hs,
                                 start=True, stop=False, tile_position=tp)
                nc.tensor.matmul(bankZ[sl, :], OHZ[xsl, q, :], xs,
                                 start=False, stop=True, tile_position=xtp)
                nc.tensor.matmul(bankN[sl, :], WN[sl, :], hs,
                                 start=True, stop=True, tile_position=tp)
                nc.tensor.matmul(bankX[sl, :], OHN[xsl, q, :], xs,
                                 start=True, stop=True, tile_position=xtp)
            SR = work.tile([128, HG], f16, tag=f"SR{g}")
            SZ = work.tile([128, HG], f16, tag=f"SZ{g}")
            T1 = work.tile([128, HG], f16, tag=f"T1{g}")
            T2 = work.tile([128, HG], f16, tag=f"T2{g}")
            NN = work.tile([128, HG], f16, tag=f"NN{g}")
            U = work.tile([128, HG], f16, tag=f"U{g}")
            V = work.tile([128, HG], f16, tag=f"V{g}")
            nc.scalar.activation(SR[:], bankR[:], AF.Sigmoid, bias=BR[:])
            nc.scalar.activation(SZ[:], bankZ[:], AF.Sigmoid, bias=BZ[:])
            # T1 = (hn + b_hh_n) * r
            nc.vector.scalar_tensor_tensor(T1[:], bankN[:], BNH[:], SR[:],
                                           op0=OP.add, op1=OP.mult)
            # T2 = T1 + xn
            nc.vector.tensor_add(T2[:], T1[:], bankX[:])
            # n = tanh(T2 + b_ih_n)
            nc.scalar.activation(NN[:], T2[:], AF.Tanh, bias=BNI[:])
            # h' = n + z*(h - n)
            nc.vector.tensor_sub(U[:], H[:, fd], NN[:])
            nc.vector.tensor_mul(V[:], SZ[:], U[:])
            nc.vector.tensor_add(H[:, fd], NN[:], V[:])

        def body(blk):
            xsb = X[:, blk, :]
            for q in range(UNROLL):
                for g in range(NGROUP):
                    step(q, xsb, g)

        if NBLK == 1:
            body(0)
        else:
            with tc.For_i(0, NBLK, 1,
                          hint_engines=(mybir.EngineType.PE,)) as i:
                body(bass.ds(i, 1))

        # Final FC: out[o, b] = sum_k fc_w[o, k] h[b, k] + fc_b[o]
        for oh in range(2):
            osl = slice(oh * 128, (oh + 1) * 128)
            fc_u = psum.tile([128, HB], f32, tag="bankR0")
            fc_v = psum.tile([128, HB], f32, tag="bankZ0")
            nc.tensor.matmul(fc_u[:], FCW[0:64, osl], H[0:64, :],
                             start=True, stop=True, tile_position=(0, 0))
            nc.tensor.matmul(fc_v[:], FCW[64:128, osl], H[64:128, :],
                             start=True, stop=True, tile_position=(64, 0))
            Ou = work.tile([128, HB], f32, tag="Ou")
            Ov = work.tile([128, HB], f32, tag="Ov")
            nc.scalar.activation(Ou[:], fc_u[:], AF.Identity,
                                 bias=FCB[:, oh:oh + 1])
            nc.scalar.activation(Ov[:], fc_v[:], AF.Identity,
                                 bias=FCB[:, oh:oh + 1])
            nc.gpsimd.dma_start(d_out[osl, 0:HB], Ou[:])
            nc.gpsimd.dma_start(d_out[osl, HB:BC], Ov[:])

    nc.compile()
    return nc


def _host_inputs(x, w_ih, w_hh, b_ih, b_hh, fc_w, fc_b):
    """Build the per-core in_maps (numpy, laid out exactly as SBUF tiles)."""
    f16 = np.float16
    f32 = np.float32
    x = np.asarray(x, f32)
    w_ih = np.asarray(w_ih, f32)
    w_hh = np.asarray(w_hh, f32)
    b_ih = np.asarray(b_ih, f32)
    b_hh = np.asarray(b_hh, f32)
    fc_w = np.asarray(fc_w, f32)
    fc_b = np.asarray(fc_b, f32)

    eye = np.eye(UNROLL, dtype=f32)

    def oh(seg):
        w = w_ih[seg, 0]
        o = np.einsum("pq,m->pqm", eye, w)          # [64, UNROLL, 64]
        return np.concatenate([o, o], 0).astype(f16)  # [128, UNROLL, 64]

    def wstack(seg):
        t = w_hh[seg, :].T                            # [64(k), 64(m)]
        return np.vstack([t, t]).astype(f16)

    def btile(v):
        return np.tile(v.reshape(-1, 1), (2, 1)).astype(f32)  # [128, 1]

    shared = {
        "wr": wstack(slice(0, 64)),
        "wz": wstack(slice(64, 128)),
        "wn": wstack(slice(128, 192)),
        "ohr": oh(slice(0, 64)),
        "ohz": oh(slice(64, 128)),
        "ohn": oh(slice(128, 192)),
        "br": btile(b_ih[0:64] + b_hh[0:64]),
        "bz": btile(b_ih[64:128] + b_hh[64:128]),
        "bnh": btile(b_hh[128:192]),
        "bni": btile(b_ih[128:192]),
        "fcw": np.vstack([fc_w.T, fc_w.T]).astype(f16),  # [128, 256]
        "fcb": np.stack([fc_b[0:128], fc_b[128:256]], 1).astype(f32),
    }

    in_maps = []
    for c in range(NCORES):
        xs = x[c * BC:(c + 1) * BC, :T, 0]            # [BC b, T t]
        xT = np.ascontiguousarray(xs.T)               # [T, BC]
        xr = xT.reshape(NBLK, UNROLL, BC)             # [blk, p, b]
        lo = xr[:, :, 0:HB].transpose(1, 0, 2)        # [64, blk, HB]
        hi = xr[:, :, HB:BC].transpose(1, 0, 2)
        Xh = np.ascontiguousarray(
            np.concatenate([lo, hi], 0)).astype(f16)  # [128, blk, HB]
        m = dict(shared)
        m["xt"] = Xh
        in_maps.append(m)
    return in_maps


def _run(in_maps, trace=False):
    from concourse import bass_utils
    if "nc" not in _CACHE:
        _CACHE["nc"] = _build()
    nc = _CACHE["nc"]
    res = bass_utils.run_bass_kernel_spmd(
        nc, in_maps, core_ids=list(range(NCORES)), trace=trace)
    return res


def kernel(**inputs):
    in_maps = _host_inputs(**inputs)
    res = _run(in_maps, trace=False)
    out = np.empty([B, OUT], np.float32)
    for c in range(NCORES):
        out[c * BC:(c + 1) * BC, :] = res.results[c]["out"].T
    return out



# revision 20
# speedup vs baseline: 220.1090x; 220.1090x over previous
"""GRU decoder kernel for Trainium2 (8 NeuronCores, data-parallel over batch).

Math (PyTorch GRU, gate order r,z,n), per batch element:
    gx_t = x_t * w_ih + b_ih              (input dim == 1 -> rank-1)
    gh_t = h_{t-1} @ w_hh.T + b_hh
    r = sigmoid(gx_r + gh_r); z = sigmoid(gx_z + gh_z)
    n = tanh(gx_n + b_ih_n + r * (gh_n + b_hh_n))
    h_t = (1-z)*n + z*h_{t-1}
    out = h_T @ fc_w.T + fc_b

Device layout (per core, B_c = 1024 batch):
  - batch 0-511 ("u") gates on PSUM partitions 0-63 (PE columns 0-63),
    batch 512-1023 ("v") on partitions 64-127 (columns 64-127); the two
    column-groups run concurrently.
  - Combined rhs tiles: HXu = [H_u (rows 0-63); X_u block (rows 64-127)],
    HXv = [X_v block (rows 0-63); H_v (rows 64-127)].  r/z gates are ONE
    K=128 matmul per half: lhsT = [w_hh gate block ; onehot(q) x w_ih]
    (combined weight tile per (gate, half, q)).  h+x accumulate inside
    the PE array -> single-shot PSUM writes, no cross-quadrant
    accumulation groups.
  - n-gate: hn via K=64 matmul from HX rows holding H; xn via K=64
    one-hot matmul from an UNSWAPPED X copy at quadrants (0,0)/(64,64);
    T1 = (hn+b_hh_n)*r is added into the xn bank by K=64 identity
    matmuls at the same quadrants (PSUM accumulate).
  - biases folded into activation bias / STT scalar.
  - fp16 SBUF tensors, fp32 PSUM accumulation.
"""

import os
import sys

sys.path.insert(0, "/opt/trn_rl_repo")

import numpy as np
from contextlib import ExitStack

HIDDEN = 64
OUT = 256
B = 8192
T = int(os.environ.get("GRU_T", 1024))
NCORES = 8
BC = B // NCORES          # 1024 batch per core
HB = BC // 2              # 512 batch per partition-half
UNROLL = 64               # steps per loop body (one-hot q index is static)
NGROUP = int(os.environ.get("GRU_NGROUP", 2))  # phase-shifted batch groups
NBLK = T // UNROLL        # number of 64-step blocks
GPS_SUB = os.environ.get("GRU_GPS", "0") == "1"  # sub on GpSimd

_CACHE = {}


def _build():
    import concourse.bass as bass
    import concourse.tile as tile
    from concourse import bacc, mybir

    f16 = mybir.dt.float16
    f32 = mybir.dt.float32
    AF = mybir.ActivationFunctionType
    OP = mybir.AluOpType

    nc = bacc.Bacc("TRN2", target_bir_lowering=False, debug=False,
                   num_devices=NCORES)

    # xs: swapped block layout (rows 0-63 = v's x, rows 64-127 = u's x)
    # xu: unswapped (rows 0-63 = u's x, rows 64-127 = v's x)
    d_xs = nc.dram_tensor("xs", [128, NBLK, HB], f16, kind="ExternalInput").ap()
    d_xu = nc.dram_tensor("xu", [128, NBLK, HB], f16, kind="ExternalInput").ap()
    d_wur = nc.dram_tensor("wur", [128, UNROLL, 64], f16, kind="ExternalInput").ap()
    d_wvr = nc.dram_tensor("wvr", [128, UNROLL, 64], f16, kind="ExternalInput").ap()
    d_wuz = nc.dram_tensor("wuz", [128, UNROLL, 64], f16, kind="ExternalInput").ap()
    d_wvz = nc.dram_tensor("wvz", [128, UNROLL, 64], f16, kind="ExternalInput").ap()
    d_wn = nc.dram_tensor("wn", [128, 64], f16, kind="ExternalInput").ap()
    d_ohn = nc.dram_tensor("ohn", [128, UNROLL, 64], f16, kind="ExternalInput").ap()
    d_eye = nc.dram_tensor("eye", [128, 64], f16, kind="ExternalInput").ap()
    d_br = nc.dram_tensor("br", [128, 1], f32, kind="ExternalInput").ap()
    d_bz = nc.dram_tensor("bz", [128, 1], f32, kind="ExternalInput").ap()
    d_bnh = nc.dram_tensor("bnh", [128, 1], f32, kind="ExternalInput").ap()
    d_bni = nc.dram_tensor("bni", [128, 1], f32, kind="ExternalInput").ap()
    d_fcw = nc.dram_tensor("fcw", [128, OUT], f16, kind="ExternalInput").ap()
    d_fcb = nc.dram_tensor("fcb", [128, 2], f32, kind="ExternalInput").ap()
    d_out = nc.dram_tensor("out", [OUT, BC], f32, kind="ExternalOutput").ap()

    with tile.TileContext(nc) as tc, ExitStack() as ctx:
        singles = ctx.enter_context(tc.tile_pool(name="singles", bufs=1))
        work = ctx.enter_context(tc.tile_pool(name="work", bufs=4))
        psum = ctx.enter_context(tc.tile_pool(name="psum", bufs=1, space="PSUM"))

        XU = singles.tile([128, NBLK, HB], f16)
        WUR = singles.tile([128, UNROLL, 64], f16)
        WVR = singles.tile([128, UNROLL, 64], f16)
        WUZ = singles.tile([128, UNROLL, 64], f16)
        WVZ = singles.tile([128, UNROLL, 64], f16)
        WN = singles.tile([128, 64], f16)
        OHN = singles.tile([128, UNROLL, 64], f16)
        EYE = singles.tile([128, 64], f16)
        BR = singles.tile([128, 1], f32)
        BZ = singles.tile([128, 1], f32)
        BNH = singles.tile([128, 1], f32)
        BNI = singles.tile([128, 1], f32)
        FCW = singles.tile([128, OUT], f16)
        FCB = singles.tile([128, 2], f32)
        HXu = singles.tile([128, HB], f16)
        HXv = singles.tile([128, HB], f16)

        for dst, src in ((XU, d_xu),
                         (WUR, d_wur), (WVR, d_wvr),
                         (WUZ, d_wuz), (WVZ, d_wvz),
                         (WN, d_wn), (OHN, d_ohn), (EYE, d_eye),
                         (BR, d_br), (BZ, d_bz), (BNH, d_bnh), (BNI, d_bni),
                         (FCW, d_fcw), (FCB, d_fcb)):
            nc.gpsimd.dma_start(dst[:], src[:])
        nc.vector.memset(HXu[:], 0.0)
        nc.vector.memset(HXv[:], 0.0)

        HG = HB // NGROUP   # free-dim width per pipelined batch group
        GS = [slice(g * HG, (g + 1) * HG) for g in range(NGROUP)]
        u, v = slice(0, 64), slice(64, 128)

        def step(q, xun):
            bankR = [psum.tile([128, HG], f32, tag=f"bankR{g}", name=f"bankR{g}")
                     for g in range(NGROUP)]
            bankZ = [psum.tile([128, HG], f32, tag=f"bankZ{g}", name=f"bankZ{g}")
                     for g in range(NGROUP)]
            bankN = [psum.tile([128, HG], f32, tag=f"bankN{g}", name=f"bankN{g}")
                     for g in range(NGROUP)]
            bankX = [psum.tile([128, HG], f32, tag=f"bankX{g}", name=f"bankX{g}")
                     for g in range(NGROUP)]

            # r and z gates: one K=128 fused h+x matmul per half per group
            for g in range(NGROUP):
                nc.tensor.matmul(bankR[g][u, :], WUR[:, q, :], HXu[:, GS[g]],
                                 start=True, stop=True, tile_position=(0, 0))
            for g in range(NGROUP):
                nc.tensor.matmul(bankR[g][v, :], WVR[:, q, :], HXv[:, GS[g]],
                                 start=True, stop=True, tile_position=(0, 64))
            for g in range(NGROUP):
                nc.tensor.matmul(bankZ[g][u, :], WUZ[:, q, :], HXu[:, GS[g]],
                                 start=True, stop=True, tile_position=(0, 0))
            for g in range(NGROUP):
                nc.tensor.matmul(bankZ[g][v, :], WVZ[:, q, :], HXv[:, GS[g]],
                                 start=True, stop=True, tile_position=(0, 64))
            # n-gate h-part (K=64 from the H rows of HXu/HXv)
            for g in range(NGROUP):
                nc.tensor.matmul(bankN[g][u, :], WN[u, :], HXu[u, GS[g]],
                                 start=True, stop=True, tile_position=(0, 0))
            for g in range(NGROUP):
                nc.tensor.matmul(bankN[g][v, :], WN[v, :], HXv[v, GS[g]],
                                 start=True, stop=True, tile_position=(64, 64))
            # n-gate x-part from UNSWAPPED X at the EYE quadrants
            for g in range(NGROUP):
                nc.tensor.matmul(bankX[g][u, :], OHN[u, q, :], xun[u, :, GS[g]],
                                 start=True, stop=False, tile_position=(0, 0))
            for g in range(NGROUP):
                nc.tensor.matmul(bankX[g][v, :], OHN[v, q, :], xun[v, :, GS[g]],
                                 start=True, stop=False, tile_position=(64, 64))

            SR = [work.tile([128, HG], f16, tag=f"SR{g}", name=f"SR{g}")
                  for g in range(NGROUP)]
            SZ = [work.tile([128, HG], f16, tag=f"SZ{g}", name=f"SZ{g}")
                  for g in range(NGROUP)]
            T1 = [work.tile([128, HG], f16, tag=f"T1{g}", name=f"T1{g}")
                  for g in range(NGROUP)]
            NN = [work.tile([128, HG], f16, tag=f"NN{g}", name=f"NN{g}")
                  for g in range(NGROUP)]
            U = [work.tile([128, HG], f16, tag=f"U{g}", name=f"U{g}")
                 for g in range(NGROUP)]
            V = [work.tile([128, HG], f16, tag=f"V{g}", name=f"V{g}")
                 for g in range(NGROUP)]

            for g in range(NGROUP):
                nc.scalar.activation(SR[g][:], bankR[g][:], AF.Sigmoid,
                                     bias=BR[:])
                nc.scalar.activation(SZ[g][:], bankZ[g][:], AF.Sigmoid,
                                     bias=BZ[:])
            for g in range(NGROUP):
                # T1 = (hn + b_hh_n) * r
                nc.vector.scalar_tensor_tensor(T1[g][:], bankN[g][:], BNH[:],
                                               SR[g][:],
                                               op0=OP.add, op1=OP.mult)
            for g in range(NGROUP):
                # bankX += T1 (identity matmuls at (0,0)/(64,64) close it)
                nc.tensor.matmul(bankX[g][u, :], EYE[u, :], T1[g][u, :],
                                 start=False, stop=True, tile_position=(0, 0))
                nc.tensor.matmul(bankX[g][v, :], EYE[v, :], T1[g][v, :],
                                 start=False, stop=True,
                                 tile_position=(64, 64))
            for g in range(NGROUP):
                # n = tanh(xn + T1 + b_ih_n)
                nc.scalar.activation(NN[g][:], bankX[g][:], AF.Tanh,
                                     bias=BNI[:])
            sub_eng = nc.gpsimd if GPS_SUB else nc.vector
            for g in range(NGROUP):
                # h' = n + z*(h - n); H lives split across HXu/HXv halves
                sub_eng.tensor_sub(U[g][u, :], HXu[u, GS[g]], NN[g][u, :])
                sub_eng.tensor_sub(U[g][v, :], HXv[v, GS[g]], NN[g][v, :])
                nc.vector.tensor_mul(V[g][:], SZ[g][:], U[g][:])
                nc.vector.tensor_add(HXu[u, GS[g]], NN[g][u, :], V[g][u, :])
                nc.vector.tensor_add(HXv[v, GS[g]], NN[g][v, :], V[g][v, :])

        def body(blk):
            # refill the X rows of the combined tiles for this block
            # (dynamic-offset DMA straight from DRAM; the DVE mis-indexes
            # register-offset APs inside For_i)
            nc.gpsimd.dma_start(HXu[v, :], d_xs[v, blk, :])
            nc.gpsimd.dma_start(HXv[u, :], d_xs[u, blk, :])
            xun = XU[:, blk, :]
            for q in range(UNROLL):
                step(q, xun)

        if NBLK == 1:
            body(slice(0, 1))
        elif os.environ.get("GRU_NOFORI", "1") == "1":
            for b in range(NBLK):
                body(slice(b, b + 1))
        else:
            with tc.For_i(0, NBLK, 1,
                          hint_engines=(mybir.EngineType.PE,)) as i:
                body(bass.ds(i, 1))

        # Final FC: out[o, b] = sum_k fc_w[o, k] h[b, k] + fc_b[o]
        for oh in range(2):
            osl = slice(oh * 128, (oh + 1) * 128)
            fc_u = psum.tile([128, HB], f32, tag="bankR0")
            fc_v = psum.tile([128, HB], f32, tag="bankZ0")
            nc.tensor.matmul(fc_u[:], FCW[0:64, osl], HXu[u, :],
                             start=True, stop=True, tile_position=(0, 0))
            nc.tensor.matmul(fc_v[:], FCW[64:128, osl], HXv[v, :],
                             start=True, stop=True, tile_position=(64, 0))
            Ou = work.tile([128, HB], f32, tag="Ou")
            Ov = work.tile([128, HB], f32, tag="Ov")
            nc.scalar.activation(Ou[:], fc_u[:], AF.Identity,
                                 bias=FCB[:, oh:oh + 1])
            nc.scalar.activation(Ov[:], fc_v[:], AF.Identity,
                                 bias=FCB[:, oh:oh + 1])
            nc.gpsimd.dma_start(d_out[osl, 0:HB], Ou[:])
            nc.gpsimd.dma_start(d_out[osl, HB:BC], Ov[:])

    nc.compile()
    return nc


def _host_inputs(x, w_ih, w_hh, b_ih, b_hh, fc_w, fc_b):
    """Build the per-core in_maps (numpy, laid out exactly as SBUF tiles)."""
    f16 = np.float16
    f32 = np.float32
    x = np.asarray(x, f32)
    w_ih = np.asarray(w_ih, f32)
    w_hh = np.asarray(w_hh, f32)
    b_ih = np.asarray(b_ih, f32)
    b_hh = np.asarray(b_hh, f32)
    fc_w = np.asarray(fc_w, f32)
    fc_b = np.asarray(fc_b, f32)

    eye = np.eye(UNROLL, dtype=f32)

    def onehot(seg):
        w = w_ih[seg, 0]
        return np.einsum("pq,m->pqm", eye, w)         # [64, UNROLL, 64]

    def wrep(seg):
        t = w_hh[seg, :].T                            # [64(k), 64(m)]
        return np.broadcast_to(t[:, None, :], (64, UNROLL, 64))

    def btile(vv):
        return np.tile(vv.reshape(-1, 1), (2, 1)).astype(f32)  # [128, 1]

    rs, zs, ns = slice(0, 64), slice(64, 128), slice(128, 192)
    shared = {
        "wur": np.concatenate([wrep(rs), onehot(rs)], 0).astype(f16),
        "wvr": np.concatenate([onehot(rs), wrep(rs)], 0).astype(f16),
        "wuz": np.concatenate([wrep(zs), onehot(zs)], 0).astype(f16),
        "wvz": np.concatenate([onehot(zs), wrep(zs)], 0).astype(f16),
        "wn": np.vstack([w_hh[ns, :].T, w_hh[ns, :].T]).astype(f16),
        "ohn": np.concatenate([onehot(ns), onehot(ns)], 0).astype(f16),
        "eye": np.vstack([np.eye(64), np.eye(64)]).astype(f16),
        "br": btile(b_ih[0:64] + b_hh[0:64]),
        "bz": btile(b_ih[64:128] + b_hh[64:128]),
        "bnh": btile(b_hh[128:192]),
        "bni": btile(b_ih[128:192]),
        "fcw": np.vstack([fc_w.T, fc_w.T]).astype(f16),  # [128, 256]
        "fcb": np.stack([fc_b[0:128], fc_b[128:256]], 1).astype(f32),
    }

    in_maps = []
    for c in range(NCORES):
        xs = x[c * BC:(c + 1) * BC, :T, 0]            # [BC b, T t]
        xT = np.ascontiguousarray(xs.T)               # [T, BC]
        xr = xT.reshape(NBLK, UNROLL, BC)             # [blk, p, b]
        lo = xr[:, :, 0:HB].transpose(1, 0, 2)        # [64, blk, HB]  u-half
        hi = xr[:, :, HB:BC].transpose(1, 0, 2)       # v-half
        m = dict(shared)
        m["xs"] = np.ascontiguousarray(
            np.concatenate([hi, lo], 0)).astype(f16)  # swapped
        m["xu"] = np.ascontiguousarray(
            np.concatenate([lo, hi], 0)).astype(f16)  # unswapped
        in_maps.append(m)
    return in_maps


def _run(in_maps, trace=False):
    from concourse import bass_utils
    if "nc" not in _CACHE:
        _CACHE["nc"] = _build()
    nc = _CACHE["nc"]
    res = bass_utils.run_bass_kernel_spmd(
        nc, in_maps, core_ids=list(range(NCORES)), trace=trace)
    return res


def kernel(**inputs):
    in_maps = _host_inputs(**inputs)
    res = _run(in_maps, trace=False)
    out = np.empty([B, OUT], np.float32)
    for c in range(NCORES):
        out[c * BC:(c + 1) * BC, :] = res.results[c]["out"].T
    return out
